# revision 26
# baseline (speedup 1.0000x reference)
"""Trainium2 Bass kernel for nn_MemoryCell (scatter_memory).

Full-input contract: kernel(**inputs) takes the complete (unsharded) numpy
inputs and returns the full [NB*B, H] output.

Math (B == H == 1024, NB == 5, T == 128):
    enc  = features[:, 0, :]                         # [B, H] - only slice used
    h    = states.reshape(NB, H)
    gate = sigmoid(enc @ (h + keys).T)               # [B, NB]
    pre  = (h @ Uw.T + keys @ Vw.T)[:, None, :] + (enc @ Ww.T)[None, :, :]
    cand = where(pre >= 0, pre, prelu_a * pre)
    new[i, b, j] = h[i, j] + gate[j, i] * cand[i, b, j]   # B==H broadcast quirk
    out  = sign(new) with exact zeros -> +1, reshaped [NB*B, H]

Sharding: split the feature/column axis j (H=1024) into 8 shards of 128
(one per core).  Each core computes ew = enc @ Ww[js].T for all b plus its
own gate[js]/huv[js] slices, so nothing needs a collective.

Precision: enc ships HI-ONLY fp16 (2 MB/core) and the big ew matmul is a
SINGLE fp16 pass (e_hi @ W_hi) - the sign-output tolerance (rel < 2e-2 ~=
524 sign flips) makes the ~6e-4 product error harmless (~140 flips
against an fp64 oracle).  Only the gate logits z keep extra passes
(e_hi @ hk_lo and e_lo[js] @ hk_hi) since sigmoid sensitivity near z~0
is the dominant flip source.

DMA: ALL inputs stream on the sync hardware-DGE queue in priority order
(identity -> one combined 1.2 MB weights/stationary pack -> enc in 4
b-quarters), so nothing waits behind a slow queue.  The enc columns are
rolled per-core so each core's own 128 j-columns sit first in quarter 0:
the z-series moving operand is a plain [0:128] slice of the enc stream
(same program on every core) and the host un-rolls the output columns.

Tail: sign(gate*ew + gate*huv + h) == (ew >= c) with
c = -(gate*huv + h) / max(gate, 1e-30), a per-partition scalar -> ONE
compare per (i, quarter): DVE does i=0,1,2 as is_ge straight from PSUM;
ScalarE copies the quarter to SBUF (table-free ACT Copy) and GpSimd
(which cannot read PSUM) does i=3,4 from the copy.  gate==0 underflow
reproduces sign(h) exactly through the clamp.  int8 {1,0} output; the
host re-expands to +-1.0 fp32.

PSUM gotcha (measured): matmul start=True clears the whole target tile,
not just the written column region - each accumulation series has exactly
one start (its first MM) and one stop (its last).
"""

import os
import numpy as np

H = 1024
NB = 5
B = 1024
NCORES = 8
JS = H // NCORES          # 128 columns per core
KC = H // 128             # 8 contraction chunks
NQ = 4                    # b processed in quarters (PSUM + tail pipelining)
QB = B // NQ
SWH = 37                  # S_hi width: hk_hi@0, h_hi@32
SWK = 37                  # S_lk width: hk_lo@0, keys_hi@32
KW = 3 * JS + SWH + SWK   # per-chunk width of the pack: 458
N_WARM = 14               # PE clock-ramp transposes (bridge until data)

# per-chunk offsets inside the pack: [ut | vt | wt | ssh | sslk]
O_UT, O_VT, O_WT, O_SH, O_SK = (0, JS, 2 * JS, 3 * JS, 3 * JS + SWH)

_NC_CACHE = {}


def _build_nc(general_prelu: bool):
    from concourse import bacc, mybir
    import concourse.tile as tile

    f32 = mybir.dt.float32
    f16 = mybir.dt.float16
    i8 = mybir.dt.int8
    AF = mybir.ActivationFunctionType
    ALU = mybir.AluOpType

    nc = bacc.Bacc("TRN2", debug=False, num_devices=NCORES)

    big = nc.dram_tensor("big", [128, KC * KW], f16, kind="ExternalInput").ap()
    # ihs cols: 0:128 identity, 128:133 h[:,js].T, 133:134 prelu_a[js]
    hs_f = NB + (1 if general_prelu else 0)
    ihs = nc.dram_tensor("ihs", [128, 128 + hs_f], f32,
                         kind="ExternalInput").ap()
    ehq = [nc.dram_tensor(f"ehq{q}", [128, KC * QB], f16,
                          kind="ExternalInput").ap() for q in range(NQ)]
    outv = nc.dram_tensor("outv", [128, 3, B], i8, kind="ExternalOutput").ap()
    outs_ = nc.dram_tensor("outs", [128, 2, B], i8,
                           kind="ExternalOutput").ap()

    with tile.TileContext(nc) as tc:
        with (
            tc.tile_pool(name="res", bufs=1) as res,
            tc.tile_pool(name="pew", bufs=4, space="PSUM") as pew,
            tc.tile_pool(name="psmall", bufs=1, space="PSUM") as psmall,
        ):
            # ---- input DMAs: ONE hwdge queue (sync), priority order ----
            ihs_t = res.tile([128, 128 + hs_f], f32, name="ihs_t")
            nc.sync.dma_start(ihs_t, ihs)
            idn_t = ihs_t[:, 0:128]
            hsm_t = ihs_t[:, 128:128 + hs_f]
            big_t = res.tile([128, KC * KW], f16, name="big_t")
            nc.sync.dma_start(big_t, big)
            ehq_t = [res.tile([128, KC, QB], f16, name=f"ehq_t{q}",
                              tag=f"ehq{q}") for q in range(NQ)]
            # split enc quarters across BOTH hwdge queues for bandwidth
            for q in range(NQ):
                eng = nc.sync if q % 2 == 0 else nc.scalar
                eng.dma_start(ehq_t[q], ehq[q])

            def bigsl(k, off, w):
                base = k * KW + off
                return big_t[:, base:base + w]

            # ---- PE ramp warm-up on the identity ----
            psum_warm = psmall.tile([128, 128], f32, name="psum_warm",
                                    padded_shape=[128, 512])
            for _ in range(N_WARM):
                nc.tensor.transpose(psum_warm, idn_t, idn_t)

            # ---- small series: [hu+kv | z] in one [SWH, 256] PSUM tile --
            # cols 0:128 rows 32:37 = hu+kv (kv FOLDED onto hu's rows via
            # the keys@32 stationary); cols 128:256 rows 0:5 = z.  Garbage
            # off-blocks ignored.  Exactly one start / one stop (PSUM
            # gotcha).  Phase 1 (big-pack only) runs ~2us before enc q0.
            psum_gv = psmall.tile([SWH, 2 * JS], f32, name="psum_gv",
                                  padded_shape=[SWH, 512])
            zcols = psum_gv[:, JS:2 * JS]
            ehj = ehq_t[0][:, :, 0:JS]
            for k in range(KC):
                nc.tensor.matmul(psum_gv[:, 0:JS], lhsT=bigsl(k, O_SH, SWH),
                                 rhs=bigsl(k, O_UT, JS),
                                 start=(k == 0), stop=False)
            for k in range(KC):
                nc.tensor.matmul(psum_gv[:, 0:JS], lhsT=bigsl(k, O_SK, SWK),
                                 rhs=bigsl(k, O_VT, JS),
                                 start=False, stop=False)
            for k in range(KC):
                nc.tensor.matmul(zcols, lhsT=bigsl(k, O_SH, SWH),
                                 rhs=ehj[:, k, :], start=False, stop=False)
            for k in range(KC):
                nc.tensor.matmul(zcols, lhsT=bigsl(k, O_SK, SWK),
                                 rhs=ehj[:, k, :], start=False,
                                 stop=(k == KC - 1))

            # sigmoid + huv straight off psum_gv rows, then one transpose
            # flips both to j-on-partitions
            gh_sb = res.tile([128, 128], f32, name="gh_sb")
            nc.gpsimd.memset(gh_sb, 0.0)
            nc.scalar.activation(gh_sb[0:NB, :], psum_gv[0:NB, JS:2 * JS],
                                 AF.Sigmoid)
            nc.vector.tensor_copy(out=gh_sb[32:32 + NB, :],
                                  in_=psum_gv[32:32 + NB, 0:JS])

            # ---- ew = e_hi @ Ww[js]_hi.T, one fp16 pass, b in quarters ----
            # separate per-engine output tiles: cross-engine writes to one
            # tile would WAW-serialize the whole tail
            o_dve = res.tile([128, 3, B], i8, name="o_dve")
            o_se = res.tile([128, 2, B], i8, name="o_se")
            vecs = gate = bias3 = c_sb = huv = None
            for q in range(NQ):
                pew_t = pew.tile([128, QB], f32, name="pew_t", tag="ew",
                                 padded_shape=[128, 512])
                for k in range(KC):
                    nc.tensor.matmul(pew_t, lhsT=bigsl(k, O_WT, JS),
                                     rhs=ehq_t[q][:, k, :],
                                     start=(k == 0), stop=(k == KC - 1))
                if q == 0:
                    # transpose gate/huv between ew quarters; short DVE
                    # chain builds bias3 / negb3 while quarter 1 streams
                    psum_gh = psmall.tile([128, 128], f32, name="psum_gh",
                                          padded_shape=[128, 512])
                    nc.tensor.transpose(psum_gh, gh_sb, idn_t)
                    # vecs cols: 0:5 gate, 5:10 bias3, 10:15 negb3, 15:20 huv
                    vecs = res.tile([128, 20], f32, name="vecs")
                    gate = vecs[:, 0:NB]
                    bias3 = vecs[:, 5:5 + NB]
                    negb3 = vecs[:, 10:10 + NB]
                    huv = vecs[:, 15:15 + NB]
                    nc.vector.tensor_copy(out=gate, in_=psum_gh[:, 0:NB])
                    nc.vector.tensor_tensor(bias3, gate,
                                            psum_gh[:, 32:32 + NB], ALU.mult)
                    nc.vector.tensor_tensor(bias3, bias3, hsm_t[:, 0:NB],
                                            ALU.add)
                    nc.vector.tensor_scalar_mul(negb3, bias3, -1.0)
                    if general_prelu:
                        nc.vector.tensor_copy(out=huv,
                                              in_=psum_gh[:, 32:32 + NB])
                qs = slice(q * QB, (q + 1) * QB)

                def dst(i):
                    if i < 3:
                        return o_dve[:, i, qs]
                    return o_se[:, i - 3, qs]

                if general_prelu:
                    # generic PReLU path (prelu_a != 1): rebuild cand
                    a_col = hsm_t[:, NB:NB + 1]
                    for i in range(NB):
                        pre = res.tile([128, QB], f32, name="pre", tag="pre",
                                       bufs=2)
                        nc.vector.tensor_scalar_add(pre, pew_t, huv[:, i:i + 1])
                        mx = res.tile([128, QB], f32, name="mx", tag="mx",
                                      bufs=2)
                        nc.vector.tensor_scalar_max(mx, pre, 0.0)
                        mn = res.tile([128, QB], f32, name="mn", tag="mn",
                                      bufs=2)
                        nc.vector.tensor_scalar_min(mn, pre, 0.0)
                        cand = res.tile([128, QB], f32, name="cand",
                                        tag="cand", bufs=2)
                        nc.vector.scalar_tensor_tensor(
                            cand, in0=mn, scalar=a_col, in1=mx,
                            op0=ALU.mult, op1=ALU.add)
                        nc.scalar.activation(
                            dst(i), cand, AF.Sign, bias=hsm_t[:, i:i + 1],
                            scale=gate[:, i:i + 1])
                else:
                    # DVE: is_ge from PSUM (i<3); ScalarE: ACT Sign (i>=3)
                    # (GpSimd is far too slow for elementwise work)
                    for i in (3, 4):
                        nc.scalar.activation(
                            dst(i), pew_t, AF.Sign, bias=bias3[:, i:i + 1],
                            scale=gate[:, i:i + 1])
                    for i in (0, 1, 2):
                        # (ew*gate_i) >= -bias3_i  <=>  sign(new) >= 0
                        nc.vector.tensor_scalar(
                            dst(i), pew_t, gate[:, i:i + 1],
                            negb3[:, i:i + 1], ALU.mult, ALU.is_ge)
                if q == 1:
                    nc.sync.dma_start(outv[:, :, 0:B // 2],
                                      o_dve[:, :, 0:B // 2])
                    nc.scalar.dma_start(outs_[:, :, 0:B // 2],
                                        o_se[:, :, 0:B // 2])
                elif q == NQ - 1:
                    nc.sync.dma_start(outv[:, :, B // 2:], o_dve[:, :, B // 2:])
                    nc.scalar.dma_start(outs_[:, :, B // 2:],
                                        o_se[:, :, B // 2:])

    nc.compile()
    return nc


def _get_nc(general_prelu: bool):
    nc = _NC_CACHE.get(general_prelu)
    if nc is None:
        nc = _build_nc(general_prelu)
        _NC_CACHE[general_prelu] = nc
    return nc


def _c32(a):
    return np.ascontiguousarray(a, dtype=np.float32)


def _packT(mat_t):
    # [H, F] (contraction-major rows) -> [128, KC, F]
    F = mat_t.shape[1]
    return np.ascontiguousarray(
        mat_t.reshape(KC, 128, F).transpose(1, 0, 2))


def _split16(a):
    hi = a.astype(np.float16)
    lo = (a - hi.astype(np.float32)).astype(np.float16)
    return hi, lo


def kernel(features, states, Uw, Vw, Ww, keys, prelu_a):
    from concourse import bass_utils

    features = np.asarray(features)
    states = np.asarray(states, dtype=np.float32)
    Uw = np.asarray(Uw, dtype=np.float32)
    Vw = np.asarray(Vw, dtype=np.float32)
    Ww = np.asarray(Ww, dtype=np.float32)
    keys = np.asarray(keys, dtype=np.float32)
    prelu_a = np.asarray(prelu_a, dtype=np.float32)

    enc = np.ascontiguousarray(features[:, 0, :], dtype=np.float32)  # [B, H]
    h = states.reshape(NB, H)
    hk = h + keys

    general_prelu = not np.all(prelu_a == 1.0)
    nc = _get_nc(general_prelu)

    e_hi, _ = _split16(enc)
    ehT = _packT(np.ascontiguousarray(e_hi.T))       # [128, KC, B] f16

    hk_hi, hk_lo = _split16(hk)
    h_hi, _ = _split16(h)
    k_hi, _ = _split16(keys)
    sshA = np.zeros((128, KC, SWH), dtype=np.float16)
    sshA[:, :, 0:NB] = _packT(hk_hi.T)
    sshA[:, :, 32:32 + NB] = _packT(h_hi.T)
    sslkA = np.zeros((128, KC, SWK), dtype=np.float16)
    sslkA[:, :, 0:NB] = _packT(hk_lo.T)
    sslkA[:, :, 32:32 + NB] = _packT(k_hi.T)

    idnA = np.eye(128, dtype=np.float32)

    in_maps = []
    for c in range(NCORES):
        js = slice(c * JS, (c + 1) * JS)
        ehR = np.roll(ehT, -c * JS, axis=2)          # own js columns first
        bigA = np.empty((128, KC, KW), dtype=np.float16)
        bigA[:, :, O_UT:O_UT + JS] = _packT(Uw[js].T.astype(np.float16))
        bigA[:, :, O_VT:O_VT + JS] = _packT(Vw[js].T.astype(np.float16))
        bigA[:, :, O_WT:O_WT + JS] = _packT(Ww[js].T.astype(np.float16))
        bigA[:, :, O_SH:O_SH + SWH] = sshA
        bigA[:, :, O_SK:O_SK + SWK] = sslkA
        hs_parts = [_c32(h[:, js].T)]
        if general_prelu:
            hs_parts.append(_c32(prelu_a[js].reshape(JS, 1)))
        m = {
            "ihs": np.ascontiguousarray(
                np.concatenate([idnA] + hs_parts, axis=1)),
            "big": np.ascontiguousarray(bigA.reshape(128, KC * KW)),
        }
        for q in range(NQ):
            m[f"ehq{q}"] = np.ascontiguousarray(
                ehR[:, :, q * QB:(q + 1) * QB].reshape(128, KC * QB))
        in_maps.append(m)

    trace = bool(int(os.environ.get("KERNEL_TRACE", "0")))
    res = bass_utils.run_bass_kernel_spmd(
        nc, in_maps, core_ids=list(range(NCORES)), trace=trace)
    kernel.last_result = res

    one = np.float32(1.0)
    neg = np.float32(-1.0)
    full = np.empty((NB, B, H), dtype=np.float32)
    view = full.reshape(NB, B, NCORES, JS)
    for c in range(NCORES):
        ov = res.results[c]["outv"]                  # [128, 3, B] is_ge
        os_ = res.results[c]["outs"]                 # [128, 2, B] Sign
        if general_prelu:
            s = np.where(np.concatenate([ov, os_], axis=1) >= 0, one, neg)
        else:
            s = np.concatenate([np.where(ov > 0, one, neg),
                                np.where(os_ >= 0, one, neg)], axis=1)
        s = np.roll(s, c * JS, axis=2)               # un-roll b columns
        view[:, :, c, :] = s.transpose(1, 2, 0)      # [NB, B, 128]
    return full.reshape(NB * B, H)


# revision 27
# speedup vs baseline: 1.1164x; 1.1164x over previous
"""Trainium2 Bass kernel for nn_MemoryCell (scatter_memory).

Full-input contract: kernel(**inputs) takes the complete (unsharded) numpy
inputs and returns the full [NB*B, H] output.

Math (B == H == 1024, NB == 5, T == 128):
    enc  = features[:, 0, :]                         # [B, H] - only slice used
    h    = states.reshape(NB, H)
    gate = sigmoid(enc @ (h + keys).T)               # [B, NB]
    pre  = (h @ Uw.T + keys @ Vw.T)[:, None, :] + (enc @ Ww.T)[None, :, :]
    cand = where(pre >= 0, pre, prelu_a * pre)
    new[i, b, j] = h[i, j] + gate[j, i] * cand[i, b, j]   # B==H broadcast quirk
    out  = sign(new) with exact zeros -> +1, reshaped [NB*B, H]

Sharding: split the feature/column axis j (H=1024) into 8 shards of 128
(one per core).  Each core computes ew = enc @ Ww[js].T for all b plus its
own gate[js]/huv[js] slices, so nothing needs a collective.

Precision: enc ships HI-ONLY fp16 (2 MB/core) and the big ew matmul is a
SINGLE fp16 pass (e_hi @ W_hi) - the sign-output tolerance (rel < 2e-2 ~=
524 sign flips) makes the ~6e-4 product error harmless (~140 flips
against an fp64 oracle).  Only the gate logits z keep extra passes
(e_hi @ hk_lo and e_lo[js] @ hk_hi) since sigmoid sensitivity near z~0
is the dominant flip source.

DMA: ALL inputs stream on the sync hardware-DGE queue in priority order
(identity -> one combined 1.2 MB weights/stationary pack -> enc in 4
b-quarters), so nothing waits behind a slow queue.  The enc columns are
rolled per-core so each core's own 128 j-columns sit first in quarter 0:
the z-series moving operand is a plain [0:128] slice of the enc stream
(same program on every core) and the host un-rolls the output columns.

Tail: sign(gate*ew + gate*huv + h) == (ew >= c) with
c = -(gate*huv + h) / max(gate, 1e-30), a per-partition scalar -> ONE
compare per (i, quarter): DVE does i=0,1,2 as is_ge straight from PSUM;
ScalarE copies the quarter to SBUF (table-free ACT Copy) and GpSimd
(which cannot read PSUM) does i=3,4 from the copy.  gate==0 underflow
reproduces sign(h) exactly through the clamp.  int8 {1,0} output; the
host re-expands to +-1.0 fp32.

PSUM gotcha (measured): matmul start=True clears the whole target tile,
not just the written column region - each accumulation series has exactly
one start (its first MM) and one stop (its last).
"""

import os
import numpy as np

H = 1024
NB = 5
B = 1024
NCORES = 8
JS = H // NCORES          # 128 columns per core
KC = H // 128             # 8 contraction chunks
NQ = 4                    # b processed in quarters (PSUM + tail pipelining)
QB = B // NQ
SWH = 37                  # S_hi width: hk_hi@0, h_hi@32
SWK = 37                  # S_lk width: hk_lo@0, keys_hi@32
KW = 3 * JS + SWH + SWK   # per-chunk width of the pack: 458
N_WARM = 14               # PE clock-ramp transposes (bridge until data)

# per-chunk offsets inside the pack: [ut | vt | wt | ssh | sslk]
O_UT, O_VT, O_WT, O_SH, O_SK = (0, JS, 2 * JS, 3 * JS, 3 * JS + SWH)

_NC_CACHE = {}


def _build_nc(general_prelu: bool):
    from concourse import bacc, mybir
    import concourse.tile as tile

    f32 = mybir.dt.float32
    f16 = mybir.dt.float16
    i8 = mybir.dt.int8
    AF = mybir.ActivationFunctionType
    ALU = mybir.AluOpType

    nc = bacc.Bacc("TRN2", debug=False, num_devices=NCORES)

    big = nc.dram_tensor("big", [128, KC * KW], f16, kind="ExternalInput").ap()
    # ihs cols: 0:128 identity, 128:133 h[:,js].T, 133:134 prelu_a[js]
    hs_f = NB + (1 if general_prelu else 0)
    ihs = nc.dram_tensor("ihs", [128, 128 + hs_f], f32,
                         kind="ExternalInput").ap()
    ehq = [nc.dram_tensor(f"ehq{q}", [128, KC * QB], f16,
                          kind="ExternalInput").ap() for q in range(NQ)]
    outv = nc.dram_tensor("outv", [128, 3, B], i8, kind="ExternalOutput").ap()
    outs_ = nc.dram_tensor("outs", [128, 2, B], i8,
                           kind="ExternalOutput").ap()

    with tile.TileContext(nc) as tc:
        with (
            tc.tile_pool(name="res", bufs=1) as res,
            tc.tile_pool(name="pew", bufs=4, space="PSUM") as pew,
            tc.tile_pool(name="psmall", bufs=1, space="PSUM") as psmall,
        ):
            # ---- input DMAs: ONE hwdge queue (sync), priority order ----
            ihs_t = res.tile([128, 128 + hs_f], f32, name="ihs_t")
            nc.sync.dma_start(ihs_t, ihs)
            idn_t = ihs_t[:, 0:128]
            hsm_t = ihs_t[:, 128:128 + hs_f]
            big_t = res.tile([128, KC * KW], f16, name="big_t")
            nc.sync.dma_start(big_t, big)
            ehq_t = [res.tile([128, KC, QB], f16, name=f"ehq_t{q}",
                              tag=f"ehq{q}") for q in range(NQ)]
            for q in range(NQ):
                nc.sync.dma_start(ehq_t[q], ehq[q])

            def bigsl(k, off, w):
                base = k * KW + off
                return big_t[:, base:base + w]

            # ---- PE ramp warm-up on the identity ----
            psum_warm = psmall.tile([128, 128], f32, name="psum_warm",
                                    padded_shape=[128, 512])
            for _ in range(N_WARM):
                nc.tensor.transpose(psum_warm, idn_t, idn_t)

            # ---- small series: [hu+kv | z] in one [SWH, 256] PSUM tile --
            # cols 0:128 rows 32:37 = hu+kv (kv FOLDED onto hu's rows via
            # the keys@32 stationary); cols 128:256 rows 0:5 = z.  Garbage
            # off-blocks ignored.  Exactly one start / one stop (PSUM
            # gotcha).  Phase 1 (big-pack only) runs ~2us before enc q0.
            psum_gv = psmall.tile([SWH, 2 * JS], f32, name="psum_gv",
                                  padded_shape=[SWH, 512])
            zcols = psum_gv[:, JS:2 * JS]
            ehj = ehq_t[0][:, :, 0:JS]
            for k in range(KC):
                nc.tensor.matmul(psum_gv[:, 0:JS], lhsT=bigsl(k, O_SH, SWH),
                                 rhs=bigsl(k, O_UT, JS),
                                 start=(k == 0), stop=False)
            for k in range(KC):
                nc.tensor.matmul(psum_gv[:, 0:JS], lhsT=bigsl(k, O_SK, SWK),
                                 rhs=bigsl(k, O_VT, JS),
                                 start=False, stop=False)
            for k in range(KC):
                nc.tensor.matmul(zcols, lhsT=bigsl(k, O_SH, SWH),
                                 rhs=ehj[:, k, :], start=False, stop=False)
            for k in range(KC):
                nc.tensor.matmul(zcols, lhsT=bigsl(k, O_SK, SWK),
                                 rhs=ehj[:, k, :], start=False,
                                 stop=(k == KC - 1))

            # sigmoid + huv straight off psum_gv rows, then one transpose
            # flips both to j-on-partitions
            gh_sb = res.tile([128, 128], f32, name="gh_sb")
            nc.gpsimd.memset(gh_sb, 0.0)
            nc.scalar.activation(gh_sb[0:NB, :], psum_gv[0:NB, JS:2 * JS],
                                 AF.Sigmoid)
            nc.vector.tensor_copy(out=gh_sb[32:32 + NB, :],
                                  in_=psum_gv[32:32 + NB, 0:JS])

            # ---- ew = e_hi @ Ww[js]_hi.T, one fp16 pass, b in quarters ----
            # separate per-engine output tiles: cross-engine writes to one
            # tile would WAW-serialize the whole tail
            o_dve = res.tile([128, 3, B], i8, name="o_dve")
            o_se = res.tile([128, 2, B], i8, name="o_se")
            vecs = gate = bias3 = c_sb = huv = None
            for q in range(NQ):
                pew_t = pew.tile([128, QB], f32, name="pew_t", tag="ew",
                                 padded_shape=[128, 512])
                for k in range(KC):
                    nc.tensor.matmul(pew_t, lhsT=bigsl(k, O_WT, JS),
                                     rhs=ehq_t[q][:, k, :],
                                     start=(k == 0), stop=(k == KC - 1))
                if q == 0:
                    # transpose gate/huv between ew quarters; short DVE
                    # chain builds bias3 / negb3 while quarter 1 streams
                    psum_gh = psmall.tile([128, 128], f32, name="psum_gh",
                                          padded_shape=[128, 512])
                    nc.tensor.transpose(psum_gh, gh_sb, idn_t)
                    # vecs cols: 0:5 gate, 5:10 bias3, 10:15 negb3, 15:20 huv
                    vecs = res.tile([128, 20], f32, name="vecs")
                    gate = vecs[:, 0:NB]
                    bias3 = vecs[:, 5:5 + NB]
                    negb3 = vecs[:, 10:10 + NB]
                    huv = vecs[:, 15:15 + NB]
                    nc.vector.tensor_copy(out=gate, in_=psum_gh[:, 0:NB])
                    nc.vector.tensor_tensor(bias3, gate,
                                            psum_gh[:, 32:32 + NB], ALU.mult)
                    nc.vector.tensor_tensor(bias3, bias3, hsm_t[:, 0:NB],
                                            ALU.add)
                    nc.vector.tensor_scalar_mul(negb3, bias3, -1.0)
                    if general_prelu:
                        nc.vector.tensor_copy(out=huv,
                                              in_=psum_gh[:, 32:32 + NB])
                qs = slice(q * QB, (q + 1) * QB)

                def dst(i):
                    if i < 3:
                        return o_dve[:, i, qs]
                    return o_se[:, i - 3, qs]

                if general_prelu:
                    # generic PReLU path (prelu_a != 1): rebuild cand
                    a_col = hsm_t[:, NB:NB + 1]
                    for i in range(NB):
                        pre = res.tile([128, QB], f32, name="pre", tag="pre",
                                       bufs=2)
                        nc.vector.tensor_scalar_add(pre, pew_t, huv[:, i:i + 1])
                        mx = res.tile([128, QB], f32, name="mx", tag="mx",
                                      bufs=2)
                        nc.vector.tensor_scalar_max(mx, pre, 0.0)
                        mn = res.tile([128, QB], f32, name="mn", tag="mn",
                                      bufs=2)
                        nc.vector.tensor_scalar_min(mn, pre, 0.0)
                        cand = res.tile([128, QB], f32, name="cand",
                                        tag="cand", bufs=2)
                        nc.vector.scalar_tensor_tensor(
                            cand, in0=mn, scalar=a_col, in1=mx,
                            op0=ALU.mult, op1=ALU.add)
                        nc.scalar.activation(
                            dst(i), cand, AF.Sign, bias=hsm_t[:, i:i + 1],
                            scale=gate[:, i:i + 1])
                else:
                    # DVE: is_ge from PSUM (i<3); ScalarE: ACT Sign (i>=3)
                    # (GpSimd is far too slow for elementwise work)
                    for i in (3, 4):
                        nc.scalar.activation(
                            dst(i), pew_t, AF.Sign, bias=bias3[:, i:i + 1],
                            scale=gate[:, i:i + 1])
                    for i in (0, 1, 2):
                        # (ew*gate_i) >= -bias3_i  <=>  sign(new) >= 0
                        nc.vector.tensor_scalar(
                            dst(i), pew_t, gate[:, i:i + 1],
                            negb3[:, i:i + 1], ALU.mult, ALU.is_ge)
                if q == 1:
                    nc.sync.dma_start(outv[:, :, 0:B // 2],
                                      o_dve[:, :, 0:B // 2])
                    nc.scalar.dma_start(outs_[:, :, 0:B // 2],
                                        o_se[:, :, 0:B // 2])
                elif q == NQ - 1:
                    nc.sync.dma_start(outv[:, :, B // 2:], o_dve[:, :, B // 2:])
                    nc.scalar.dma_start(outs_[:, :, B // 2:],
                                        o_se[:, :, B // 2:])

    nc.compile()
    return nc


def _get_nc(general_prelu: bool):
    nc = _NC_CACHE.get(general_prelu)
    if nc is None:
        nc = _build_nc(general_prelu)
        _NC_CACHE[general_prelu] = nc
    return nc


def _c32(a):
    return np.ascontiguousarray(a, dtype=np.float32)


def _packT(mat_t):
    # [H, F] (contraction-major rows) -> [128, KC, F]
    F = mat_t.shape[1]
    return np.ascontiguousarray(
        mat_t.reshape(KC, 128, F).transpose(1, 0, 2))


def _split16(a):
    hi = a.astype(np.float16)
    lo = (a - hi.astype(np.float32)).astype(np.float16)
    return hi, lo


def kernel(features, states, Uw, Vw, Ww, keys, prelu_a):
    from concourse import bass_utils

    features = np.asarray(features)
    states = np.asarray(states, dtype=np.float32)
    Uw = np.asarray(Uw, dtype=np.float32)
    Vw = np.asarray(Vw, dtype=np.float32)
    Ww = np.asarray(Ww, dtype=np.float32)
    keys = np.asarray(keys, dtype=np.float32)
    prelu_a = np.asarray(prelu_a, dtype=np.float32)

    enc = np.ascontiguousarray(features[:, 0, :], dtype=np.float32)  # [B, H]
    h = states.reshape(NB, H)
    hk = h + keys

    general_prelu = not np.all(prelu_a == 1.0)
    nc = _get_nc(general_prelu)

    e_hi, _ = _split16(enc)
    ehT = _packT(np.ascontiguousarray(e_hi.T))       # [128, KC, B] f16

    hk_hi, hk_lo = _split16(hk)
    h_hi, _ = _split16(h)
    k_hi, _ = _split16(keys)
    sshA = np.zeros((128, KC, SWH), dtype=np.float16)
    sshA[:, :, 0:NB] = _packT(hk_hi.T)
    sshA[:, :, 32:32 + NB] = _packT(h_hi.T)
    sslkA = np.zeros((128, KC, SWK), dtype=np.float16)
    sslkA[:, :, 0:NB] = _packT(hk_lo.T)
    sslkA[:, :, 32:32 + NB] = _packT(k_hi.T)

    idnA = np.eye(128, dtype=np.float32)

    in_maps = []
    for c in range(NCORES):
        js = slice(c * JS, (c + 1) * JS)
        ehR = np.roll(ehT, -c * JS, axis=2)          # own js columns first
        bigA = np.empty((128, KC, KW), dtype=np.float16)
        bigA[:, :, O_UT:O_UT + JS] = _packT(Uw[js].T.astype(np.float16))
        bigA[:, :, O_VT:O_VT + JS] = _packT(Vw[js].T.astype(np.float16))
        bigA[:, :, O_WT:O_WT + JS] = _packT(Ww[js].T.astype(np.float16))
        bigA[:, :, O_SH:O_SH + SWH] = sshA
        bigA[:, :, O_SK:O_SK + SWK] = sslkA
        hs_parts = [_c32(h[:, js].T)]
        if general_prelu:
            hs_parts.append(_c32(prelu_a[js].reshape(JS, 1)))
        m = {
            "ihs": np.ascontiguousarray(
                np.concatenate([idnA] + hs_parts, axis=1)),
            "big": np.ascontiguousarray(bigA.reshape(128, KC * KW)),
        }
        for q in range(NQ):
            m[f"ehq{q}"] = np.ascontiguousarray(
                ehR[:, :, q * QB:(q + 1) * QB].reshape(128, KC * QB))
        in_maps.append(m)

    trace = bool(int(os.environ.get("KERNEL_TRACE", "0")))
    res = bass_utils.run_bass_kernel_spmd(
        nc, in_maps, core_ids=list(range(NCORES)), trace=trace)
    kernel.last_result = res

    one = np.float32(1.0)
    neg = np.float32(-1.0)
    full = np.empty((NB, B, H), dtype=np.float32)
    view = full.reshape(NB, B, NCORES, JS)
    for c in range(NCORES):
        ov = res.results[c]["outv"]                  # [128, 3, B] is_ge
        os_ = res.results[c]["outs"]                 # [128, 2, B] Sign
        if general_prelu:
            s = np.where(np.concatenate([ov, os_], axis=1) >= 0, one, neg)
        else:
            s = np.concatenate([np.where(ov > 0, one, neg),
                                np.where(os_ >= 0, one, neg)], axis=1)
        s = np.roll(s, c * JS, axis=2)               # un-roll b columns
        view[:, :, c, :] = s.transpose(1, 2, 0)      # [NB, B, 128]
    return full.reshape(NB * B, H)


# revision 28
# speedup vs baseline: 1.1212x; 1.0042x over previous
"""Trainium2 Bass kernel for nn_MemoryCell (scatter_memory).

Full-input contract: kernel(**inputs) takes the complete (unsharded) numpy
inputs and returns the full [NB*B, H] output.

Math (B == H == 1024, NB == 5, T == 128):
    enc  = features[:, 0, :]                         # [B, H] - only slice used
    h    = states.reshape(NB, H)
    gate = sigmoid(enc @ (h + keys).T)               # [B, NB]
    pre  = (h @ Uw.T + keys @ Vw.T)[:, None, :] + (enc @ Ww.T)[None, :, :]
    cand = where(pre >= 0, pre, prelu_a * pre)
    new[i, b, j] = h[i, j] + gate[j, i] * cand[i, b, j]   # B==H broadcast quirk
    out  = sign(new) with exact zeros -> +1, reshaped [NB*B, H]

Sharding: split the feature/column axis j (H=1024) into 8 shards of 128
(one per core).  Each core computes ew = enc @ Ww[js].T for all b plus its
own gate[js]/huv[js] slices, so nothing needs a collective.

Precision: enc ships HI-ONLY fp16 (2 MB/core) and the big ew matmul is a
SINGLE fp16 pass (e_hi @ W_hi) - the sign-output tolerance (rel < 2e-2 ~=
524 sign flips) makes the ~6e-4 product error harmless (~140 flips
against an fp64 oracle).  Only the gate logits z keep extra passes
(e_hi @ hk_lo and e_lo[js] @ hk_hi) since sigmoid sensitivity near z~0
is the dominant flip source.

DMA: ALL inputs stream on the sync hardware-DGE queue in priority order
(identity -> one combined 1.2 MB weights/stationary pack -> enc in 4
b-quarters), so nothing waits behind a slow queue.  The enc columns are
rolled per-core so each core's own 128 j-columns sit first in quarter 0:
the z-series moving operand is a plain [0:128] slice of the enc stream
(same program on every core) and the host un-rolls the output columns.

Tail: sign(gate*ew + gate*huv + h) == (ew >= c) with
c = -(gate*huv + h) / max(gate, 1e-30), a per-partition scalar -> ONE
compare per (i, quarter): DVE does i=0,1,2 as is_ge straight from PSUM;
ScalarE copies the quarter to SBUF (table-free ACT Copy) and GpSimd
(which cannot read PSUM) does i=3,4 from the copy.  gate==0 underflow
reproduces sign(h) exactly through the clamp.  int8 {1,0} output; the
host re-expands to +-1.0 fp32.

PSUM gotcha (measured): matmul start=True clears the whole target tile,
not just the written column region - each accumulation series has exactly
one start (its first MM) and one stop (its last).
"""

import os
import numpy as np

H = 1024
NB = 5
B = 1024
NCORES = 8
JS = H // NCORES          # 128 columns per core
KC = H // 128             # 8 contraction chunks
NQ = 4                    # b processed in quarters (PSUM + tail pipelining)
QB = B // NQ
SWH = 37                  # S_hi width: hk_hi@0, h_hi@32
SWK = 37                  # S_lk width: hk_lo@0, keys_hi@32
KW = 3 * JS + SWH + SWK   # per-chunk width of the pack: 458
N_WARM = 14               # PE clock-ramp transposes (bridge until data)

# per-chunk offsets inside the pack: [ut | vt | wt | ssh | sslk]
O_UT, O_VT, O_WT, O_SH, O_SK = (0, JS, 2 * JS, 3 * JS, 3 * JS + SWH)

_NC_CACHE = {}


def _build_nc(general_prelu: bool):
    from concourse import bacc, mybir
    import concourse.tile as tile

    f32 = mybir.dt.float32
    f16 = mybir.dt.float16
    i8 = mybir.dt.int8
    AF = mybir.ActivationFunctionType
    ALU = mybir.AluOpType

    nc = bacc.Bacc("TRN2", debug=False, num_devices=NCORES)

    big = nc.dram_tensor("big", [128, KC * KW], f16, kind="ExternalInput").ap()
    # ihs cols: 0:128 identity, 128:133 h[:,js].T, 133:134 prelu_a[js]
    hs_f = NB + (1 if general_prelu else 0)
    ihs = nc.dram_tensor("ihs", [128, 128 + hs_f], f32,
                         kind="ExternalInput").ap()
    ehq = [nc.dram_tensor(f"ehq{q}", [128, KC * QB], f16,
                          kind="ExternalInput").ap() for q in range(NQ)]
    outv = nc.dram_tensor("outv", [128, 3, B], i8, kind="ExternalOutput").ap()
    outs_ = nc.dram_tensor("outs", [128, 2, B], i8,
                           kind="ExternalOutput").ap()

    with tile.TileContext(nc) as tc:
        with (
            tc.tile_pool(name="res", bufs=1) as res,
            tc.tile_pool(name="pew", bufs=4, space="PSUM") as pew,
            tc.tile_pool(name="psmall", bufs=1, space="PSUM") as psmall,
        ):
            # ---- input DMAs: ONE hwdge queue (sync), priority order ----
            ihs_t = res.tile([128, 128 + hs_f], f32, name="ihs_t")
            nc.sync.dma_start(ihs_t, ihs)
            idn_t = ihs_t[:, 0:128]
            hsm_t = ihs_t[:, 128:128 + hs_f]
            big_t = res.tile([128, KC * KW], f16, name="big_t")
            nc.sync.dma_start(big_t, big)
            ehq_t = [res.tile([128, KC, QB], f16, name=f"ehq_t{q}",
                              tag=f"ehq{q}") for q in range(NQ)]
            for q in range(NQ):
                nc.sync.dma_start(ehq_t[q], ehq[q])

            def bigsl(k, off, w):
                base = k * KW + off
                return big_t[:, base:base + w]

            # ---- PE ramp warm-up on the identity ----
            psum_warm = psmall.tile([128, 128], f32, name="psum_warm",
                                    padded_shape=[128, 512])
            for _ in range(N_WARM):
                nc.tensor.transpose(psum_warm, idn_t, idn_t)

            # ---- small series: [hu+kv | z] in one [SWH, 256] PSUM tile --
            # cols 0:128 rows 32:37 = hu+kv (kv FOLDED onto hu's rows via
            # the keys@32 stationary); cols 128:256 rows 0:5 = z.  Garbage
            # off-blocks ignored.  Exactly one start / one stop (PSUM
            # gotcha).  Phase 1 (big-pack only) runs ~2us before enc q0.
            psum_gv = psmall.tile([SWH, 2 * JS], f32, name="psum_gv",
                                  padded_shape=[SWH, 512])
            zcols = psum_gv[:, JS:2 * JS]
            ehj = ehq_t[0][:, :, 0:JS]
            for k in range(KC):
                nc.tensor.matmul(psum_gv[:, 0:JS], lhsT=bigsl(k, O_SH, SWH),
                                 rhs=bigsl(k, O_UT, JS),
                                 start=(k == 0), stop=False)
            for k in range(KC):
                nc.tensor.matmul(psum_gv[:, 0:JS], lhsT=bigsl(k, O_SK, SWK),
                                 rhs=bigsl(k, O_VT, JS),
                                 start=False, stop=False)
            for k in range(KC):
                nc.tensor.matmul(zcols, lhsT=bigsl(k, O_SH, SWH),
                                 rhs=ehj[:, k, :], start=False, stop=False)
            for k in range(KC):
                nc.tensor.matmul(zcols, lhsT=bigsl(k, O_SK, SWK),
                                 rhs=ehj[:, k, :], start=False,
                                 stop=(k == KC - 1))

            # sigmoid + huv straight off psum_gv rows, then one transpose
            # flips both to j-on-partitions
            gh_sb = res.tile([128, 128], f32, name="gh_sb")
            nc.gpsimd.memset(gh_sb, 0.0)
            nc.scalar.activation(gh_sb[0:NB, :], psum_gv[0:NB, JS:2 * JS],
                                 AF.Sigmoid)
            nc.vector.tensor_copy(out=gh_sb[32:32 + NB, :],
                                  in_=psum_gv[32:32 + NB, 0:JS])

            # ---- ew = e_hi @ Ww[js]_hi.T, one fp16 pass, b in quarters ----
            # separate per-engine output tiles: cross-engine writes to one
            # tile would WAW-serialize the whole tail
            o_dve = res.tile([128, 3, B], i8, name="o_dve")
            o_se = res.tile([128, 2, B], i8, name="o_se")
            vecs = gate = bias3 = c_sb = huv = None
            for q in range(NQ):
                pew_t = pew.tile([128, QB], f32, name="pew_t", tag="ew",
                                 padded_shape=[128, 512])
                for k in range(KC):
                    nc.tensor.matmul(pew_t, lhsT=bigsl(k, O_WT, JS),
                                     rhs=ehq_t[q][:, k, :],
                                     start=(k == 0), stop=(k == KC - 1))
                if q == 0:
                    # transpose gate/huv between ew quarters; short DVE
                    # chain builds bias3 / negb3 while quarter 1 streams
                    psum_gh = psmall.tile([128, 128], f32, name="psum_gh",
                                          padded_shape=[128, 512])
                    nc.tensor.transpose(psum_gh, gh_sb, idn_t)
                    # vecs cols: 0:5 gate, 5:10 bias3, 10:15 negb3, 15:20 huv
                    vecs = res.tile([128, 20], f32, name="vecs")
                    gate = vecs[:, 0:NB]
                    bias3 = vecs[:, 5:5 + NB]
                    negb3 = vecs[:, 10:10 + NB]
                    huv = vecs[:, 15:15 + NB]
                    nc.vector.tensor_copy(out=gate, in_=psum_gh[:, 0:NB])
                    nc.vector.tensor_tensor(bias3, gate,
                                            psum_gh[:, 32:32 + NB], ALU.mult)
                    nc.vector.tensor_tensor(bias3, bias3, hsm_t[:, 0:NB],
                                            ALU.add)
                    nc.vector.tensor_scalar_mul(negb3, bias3, -1.0)
                    if general_prelu:
                        nc.vector.tensor_copy(out=huv,
                                              in_=psum_gh[:, 32:32 + NB])
                qs = slice(q * QB, (q + 1) * QB)

                def dst(i):
                    if i < 3:
                        return o_dve[:, i, qs]
                    return o_se[:, i - 3, qs]

                if general_prelu:
                    # generic PReLU path (prelu_a != 1): rebuild cand
                    a_col = hsm_t[:, NB:NB + 1]
                    for i in range(NB):
                        pre = res.tile([128, QB], f32, name="pre", tag="pre",
                                       bufs=2)
                        nc.vector.tensor_scalar_add(pre, pew_t, huv[:, i:i + 1])
                        mx = res.tile([128, QB], f32, name="mx", tag="mx",
                                      bufs=2)
                        nc.vector.tensor_scalar_max(mx, pre, 0.0)
                        mn = res.tile([128, QB], f32, name="mn", tag="mn",
                                      bufs=2)
                        nc.vector.tensor_scalar_min(mn, pre, 0.0)
                        cand = res.tile([128, QB], f32, name="cand",
                                        tag="cand", bufs=2)
                        nc.vector.scalar_tensor_tensor(
                            cand, in0=mn, scalar=a_col, in1=mx,
                            op0=ALU.mult, op1=ALU.add)
                        nc.scalar.activation(
                            dst(i), cand, AF.Sign, bias=hsm_t[:, i:i + 1],
                            scale=gate[:, i:i + 1])
                else:
                    # DVE: is_ge from PSUM (i<3); ScalarE: ACT Sign (i>=3)
                    # (GpSimd is far too slow for elementwise work)
                    for i in (3, 4):
                        nc.scalar.activation(
                            dst(i), pew_t, AF.Sign, bias=bias3[:, i:i + 1],
                            scale=gate[:, i:i + 1])
                    with tc.high_priority():
                        for i in (0, 1, 2):
                            # (ew*gate_i) >= -bias3_i  <=>  sign(new) >= 0
                            nc.vector.tensor_scalar(
                                dst(i), pew_t, gate[:, i:i + 1],
                                negb3[:, i:i + 1], ALU.mult, ALU.is_ge)
                if q == 1:
                    nc.sync.dma_start(outv[:, :, 0:B // 2],
                                      o_dve[:, :, 0:B // 2])
                    nc.scalar.dma_start(outs_[:, :, 0:B // 2],
                                        o_se[:, :, 0:B // 2])
                elif q == NQ - 1:
                    nc.sync.dma_start(outv[:, :, B // 2:], o_dve[:, :, B // 2:])
                    nc.scalar.dma_start(outs_[:, :, B // 2:],
                                        o_se[:, :, B // 2:])

    nc.compile()
    return nc


def _get_nc(general_prelu: bool):
    nc = _NC_CACHE.get(general_prelu)
    if nc is None:
        nc = _build_nc(general_prelu)
        _NC_CACHE[general_prelu] = nc
    return nc


def _c32(a):
    return np.ascontiguousarray(a, dtype=np.float32)


def _packT(mat_t):
    # [H, F] (contraction-major rows) -> [128, KC, F]
    F = mat_t.shape[1]
    return np.ascontiguousarray(
        mat_t.reshape(KC, 128, F).transpose(1, 0, 2))


def _split16(a):
    hi = a.astype(np.float16)
    lo = (a - hi.astype(np.float32)).astype(np.float16)
    return hi, lo


def kernel(features, states, Uw, Vw, Ww, keys, prelu_a):
    from concourse import bass_utils

    features = np.asarray(features)
    states = np.asarray(states, dtype=np.float32)
    Uw = np.asarray(Uw, dtype=np.float32)
    Vw = np.asarray(Vw, dtype=np.float32)
    Ww = np.asarray(Ww, dtype=np.float32)
    keys = np.asarray(keys, dtype=np.float32)
    prelu_a = np.asarray(prelu_a, dtype=np.float32)

    enc = np.ascontiguousarray(features[:, 0, :], dtype=np.float32)  # [B, H]
    h = states.reshape(NB, H)
    hk = h + keys

    general_prelu = not np.all(prelu_a == 1.0)
    nc = _get_nc(general_prelu)

    e_hi, _ = _split16(enc)
    ehT = _packT(np.ascontiguousarray(e_hi.T))       # [128, KC, B] f16

    hk_hi, hk_lo = _split16(hk)
    h_hi, _ = _split16(h)
    k_hi, _ = _split16(keys)
    sshA = np.zeros((128, KC, SWH), dtype=np.float16)
    sshA[:, :, 0:NB] = _packT(hk_hi.T)
    sshA[:, :, 32:32 + NB] = _packT(h_hi.T)
    sslkA = np.zeros((128, KC, SWK), dtype=np.float16)
    sslkA[:, :, 0:NB] = _packT(hk_lo.T)
    sslkA[:, :, 32:32 + NB] = _packT(k_hi.T)

    idnA = np.eye(128, dtype=np.float32)

    in_maps = []
    for c in range(NCORES):
        js = slice(c * JS, (c + 1) * JS)
        ehR = np.roll(ehT, -c * JS, axis=2)          # own js columns first
        bigA = np.empty((128, KC, KW), dtype=np.float16)
        bigA[:, :, O_UT:O_UT + JS] = _packT(Uw[js].T.astype(np.float16))
        bigA[:, :, O_VT:O_VT + JS] = _packT(Vw[js].T.astype(np.float16))
        bigA[:, :, O_WT:O_WT + JS] = _packT(Ww[js].T.astype(np.float16))
        bigA[:, :, O_SH:O_SH + SWH] = sshA
        bigA[:, :, O_SK:O_SK + SWK] = sslkA
        hs_parts = [_c32(h[:, js].T)]
        if general_prelu:
            hs_parts.append(_c32(prelu_a[js].reshape(JS, 1)))
        m = {
            "ihs": np.ascontiguousarray(
                np.concatenate([idnA] + hs_parts, axis=1)),
            "big": np.ascontiguousarray(bigA.reshape(128, KC * KW)),
        }
        for q in range(NQ):
            m[f"ehq{q}"] = np.ascontiguousarray(
                ehR[:, :, q * QB:(q + 1) * QB].reshape(128, KC * QB))
        in_maps.append(m)

    trace = bool(int(os.environ.get("KERNEL_TRACE", "0")))
    res = bass_utils.run_bass_kernel_spmd(
        nc, in_maps, core_ids=list(range(NCORES)), trace=trace)
    kernel.last_result = res

    one = np.float32(1.0)
    neg = np.float32(-1.0)
    full = np.empty((NB, B, H), dtype=np.float32)
    view = full.reshape(NB, B, NCORES, JS)
    for c in range(NCORES):
        ov = res.results[c]["outv"]                  # [128, 3, B] is_ge
        os_ = res.results[c]["outs"]                 # [128, 2, B] Sign
        if general_prelu:
            s = np.where(np.concatenate([ov, os_], axis=1) >= 0, one, neg)
        else:
            s = np.concatenate([np.where(ov > 0, one, neg),
                                np.where(os_ >= 0, one, neg)], axis=1)
        s = np.roll(s, c * JS, axis=2)               # un-roll b columns
        view[:, :, c, :] = s.transpose(1, 2, 0)      # [NB, B, 128]
    return full.reshape(NB * B, H)
